# revision 2
# baseline (speedup 1.0000x reference)
"""Enframe kernel for Trainium2 (Bass/Tile), SPMD over 8 NeuronCores.

Problem: x (16, 4, 160000) f32 -> out (16, 8192, 309) f32 where
  out[b, c*2048 + k, f] = x[b, c, 512*f + k]   (FRAME=2048, HOP=512, 309 frames)

Pure data movement. Per (b, c) slab view the signal as X2[j, r] = x[b, c, 512*j + r]
(j in [0,312), r in [0,512)). Then out[b, c*2048 + 512*q + r, f] = X2[f + q, r].
So the output is 4 shifted copies (q = 0..3) of the transpose of X2.

On-chip layout: the TensorE transpose input uses a stride-4 free-dim AP so
SBUF partition p of the transposed tile holds output rows r = 4p..4p+3:
  T2[p, i, j] = X2[j, 4p + i]     (tile shape [128, 4, 312])
With that, the store for window-shift q is a DMA whose HBM side is fully
contiguous, iterated (p, i, f) with source T2[:, :, q:q+309].

Sharding: data-parallel over batch, 2 batches per core (per-core traffic
5.12MB loads + 20.25MB stores is the irreducible minimum).

Measured facts that shaped this kernel (HW traces, profiled runs 2026-08-10):
- exec_time = NTFF last_useful - first_useful. first_useful is pinned at the
  first DMA packet of the run (~2.5us): a 12B static "instruction"-queue
  packet that persists regardless of enable_partition_id / library loads.
  last_useful = last data packet + ~5.6us (walrus wrapper epilogue: final
  barrier + ~250 per-semaphore resets split across engines - not removable
  from kernel code). So exec ~= stream_end + 3.1us; minimizing the absolute
  time of the LAST data packet is everything.
- Per-engine DMA throughput is ~24-25GB/s (port ceiling 27.2); 16 engines
  sustain ~400-406GB/s/core mixed, but HBM READS alone cap at ~300-330KB/us,
  so a load-only ramp under-feeds the engines. Hence: stores must start
  flowing as early as possible. The first two slabs store per-q-window (the
  store dispatches as soon as its window copy lands, ~16us); later slabs
  store all 4 windows in ONE [128, 16, 309] op (2.53MB contiguous HBM,
  same 4944B packets, quarter the dispatch/sem overhead).
- DMA packet->engine assignment is round-robin with phase carried across
  ops (verified: exactly uniform per-engine counts despite non-multiple-of-16
  ops). Partition-based engine skew is impossible. The run-to-run modes
  (clean ~77.5us / one-slow-engine ~88us / global-HBM ~95-100us) are
  environmental; only the clean floor responds to kernel changes.
- Loads: jt0+jt1 of each slab load as ONE [128, 2, 512] op (partition p
  holds blocks p and 128+p; SWDGE coalesces partition pairs into 4096B
  packets). Slabs 0-3 ride the two HWDGE rings (sync/scalar) in parallel
  during the ramp - HWDGE dispatch is ~650ns/op and packets hit the wire at
  ~8.4us; remaining loads go via gpsimd SWDGE, freeing both HWDGE rings for
  stores. The 56-block jt2 tails stay on gpsimd (moving them to the HWDGE
  rings produced a NaN run once - not shipped).
- All PSUM->SBUF copies on vector: scalar ACTIVATE copies would pull a
  1.3us ACT_TABLE_LOAD into the ramp. PSUM->t2 goes per-jt through one
  [128, 4, 128] one-bank PSUM tile (4 transposes, one copy).
- insert_library_loads suppressed (kernel uses no gpsimd-library
  instruction) and enable_partition_id=False: drops preamble TENSOR_LOAD /
  table work; neither unpins first_useful but both shorten the ramp.

Timings (4-sample runs, same day): baseline [80160, 102485, 88656, 79340],
this kernel [78141, 77441, 89167, 77308]. Relative error 0.0 (exact).
"""

import numpy as np

import concourse.bacc as bacc
import concourse.bass as bass
import concourse.mybir as mybir
import concourse.tile as tile
from concourse import masks
from concourse.bass_utils import run_bass_kernel_spmd

B, C, S = 16, 4, 160000
FRAME, HOP = 2048, 512
NF = (S - FRAME) // HOP + 1          # 309 frames
NBLK = NF + FRAME // HOP - 1         # 312 blocks of 512 samples actually used
N_CORES = 8
B_PER = B // N_CORES                 # 2 batches per core
F32 = mybir.dt.float32


class _NoLibBacc(bacc.Bacc):
    """Bacc whose compile() skips gpsimd library loads (kernel uses none)."""

    def insert_library_loads(self):
        pass


def build_bass():
    nc = _NoLibBacc(None, target_bir_lowering=False, enable_partition_id=False)
    x = nc.dram_tensor("x", [B_PER, C, S], F32, kind="ExternalInput")
    out = nc.dram_tensor("out", [B_PER, C * FRAME, NF], F32, kind="ExternalOutput")

    with tile.TileContext(nc) as tc:
        with (
            tc.tile_pool(name="singles", bufs=1) as singles,
            tc.tile_pool(name="a", bufs=6) as a_pool,
            tc.tile_pool(name="a2", bufs=4) as a2_pool,
            tc.tile_pool(name="t2", bufs=6) as t2_pool,
            tc.tile_pool(name="oq", bufs=4) as oq_pool,
            tc.tile_pool(name="oqr", bufs=8) as oqr_pool,
            tc.tile_pool(name="ps", bufs=8, space=bass.MemorySpace.PSUM) as ps_pool,
        ):
            ident = singles.tile([128, 128], F32)
            nc.vector.memset(ident[:], 0.0)
            masks.make_identity(nc, ident[:], nomemset=True)

            n_main = 0
            n_store = 0
            for b in range(B_PER):
                for c in range(C):
                    slab_off = (b * C + c) * S
                    # T2[p, i, j] = X2[j, 4p + i]
                    t2 = t2_pool.tile([128, 4, NBLK], F32)

                    # jt0+jt1 in one op: am[p, k, r] = X2[128k + p, r]
                    am = a_pool.tile([128, 2, HOP], F32)
                    src = bass.AP(x, slab_off,
                                  [[HOP, 128], [128 * HOP, 2], [1, HOP]])
                    ld_eng = [nc.sync, nc.scalar][n_main % 2] if n_main < 4 \
                        else nc.gpsimd
                    n_main += 1
                    ld_eng.dma_start(out=am[:], in_=src)
                    # jt2 tail: 56 blocks
                    pj2 = NBLK - 256  # 56
                    a3 = a2_pool.tile([128, HOP], F32)
                    src2 = bass.AP(x, slab_off + 256 * HOP, [[HOP, pj2], [1, HOP]])
                    nc.gpsimd.dma_start(out=a3[:pj2], in_=src2)

                    for jt in range(3):
                        pj = 128 if jt < 2 else pj2
                        a_ap = a3[:pj2] if jt == 2 else am[:, jt]
                        pst = ps_pool.tile([128, 4, 128], F32)
                        for i in range(4):
                            nc.tensor.transpose(
                                pst[:, i, :pj], a_ap[:pj, i::4], ident[:pj, :pj]
                            )
                        nc.vector.tensor_copy(
                            out=t2[:, :, jt * 128 : jt * 128 + pj],
                            in_=pst[:, :, :pj],
                        )

                    if b == 0 and c <= 1:
                        # Ramp slabs: store each q-window as soon as its
                        # copy lands, so stores overlap the read-only phase.
                        for q in range(4):
                            oqr = oqr_pool.tile([128, 4, NF], F32)
                            nc.vector.tensor_copy(
                                out=oqr[:], in_=t2[:, :, q : q + NF]
                            )
                            dst = bass.AP(
                                out,
                                (b * C * FRAME + c * FRAME + q * HOP) * NF,
                                [[4 * NF, 128], [NF, 4], [1, NF]],
                            )
                            st_eng = [nc.scalar, nc.sync][n_store % 2]
                            n_store += 1
                            st_eng.dma_start(out=dst, in_=oqr[:])
                    else:
                        # One store op for all 4 q-windows: rows 512q+4p+i,
                        # covering the contiguous rows [c*2048, (c+1)*2048).
                        oq = oq_pool.tile([128, 4, 4, NF], F32)
                        for q in range(4):
                            nc.vector.tensor_copy(
                                out=oq[:, q], in_=t2[:, :, q : q + NF]
                            )
                        dst = bass.AP(
                            out,
                            (b * C * FRAME + c * FRAME) * NF,
                            [[4 * NF, 128], [HOP * NF, 4], [NF, 4], [1, NF]],
                        )
                        st_eng = [nc.scalar, nc.sync][n_store % 2]
                        n_store += 1
                        st_eng.dma_start(out=dst, in_=oq[:])
    nc.finalize()
    return nc


_NC_CACHE = None


def kernel(x: np.ndarray) -> np.ndarray:
    global _NC_CACHE
    if _NC_CACHE is None:
        _NC_CACHE = build_bass()
    nc = _NC_CACHE
    in_maps = [
        {"x": np.ascontiguousarray(x[i * B_PER : (i + 1) * B_PER])}
        for i in range(N_CORES)
    ]
    res = run_bass_kernel_spmd(nc, in_maps, list(range(N_CORES)))
    return np.concatenate([r["out"] for r in res.results], axis=0)


# revision 3
# speedup vs baseline: 1.1493x; 1.1493x over previous
"""Enframe kernel for Trainium2 (Bass/Tile), SPMD over 8 NeuronCores.

Problem: x (16, 4, 160000) f32 -> out (16, 8192, 309) f32 where
  out[b, c*2048 + k, f] = x[b, c, 512*f + k]   (FRAME=2048, HOP=512, 309 frames)

Pure data movement: per (b,c) slab with X2[j, r] = x[b, c, 512*j + r], the
output is 4 shifted windows (q=0..3) of the transpose of X2. TensorE
transposes with a stride-4 free-dim AP put output rows 4p..4p+3 on SBUF
partition p; stores then have 4944B-contiguous runs per partition.
Sharding: data-parallel over batch, 2 batches/core (per-core traffic
5.12MB loads + 20.25MB stores is the irreducible minimum).

Design, driven by HW traces (2026-08-10):
- exec_time = NTFF last_useful - first_useful; the window opens at the
  first DMA packet (a ~2.5us static packet that cannot be removed) and
  closes ~5.6us after the last data packet (walrus wrapper sem-reset
  epilogue, also fixed). So all that matters is the absolute time of the
  LAST data packet: exec ~= stream_end + 3.1us.
- Packet->engine assignment is round-robin with carried phase (verified
  uniform); per-engine ~24-25GB/s, ~400-410GB/s/core mixed. HBM READS
  alone cap lower (~300-330KB/us/core), so the load-only ramp under-feeds
  the engines: stores must start as early as possible.
- Loads: jt0+jt1 of each slab as ONE [128,2,512] op (SWDGE coalesces
  partition pairs to 4096B packets); slabs 0-3 ride the two HWDGE rings,
  rest on gpsimd SWDGE. 56-block jt2 tails on gpsimd.
- Slab0 skips the t2 stage entirely: its 4 store windows are built
  directly from the PSUM transpose tiles by 3 scalar segment-copies each
  (one per jt, ready as each jt's transposes land) - two fewer semaphore
  hops, first store packet ~19us. Slab1 stores per-window via scalar
  copies; ramp stores ride sync. Slabs 2-7 copy PSUM->t2 per jt
  ([128,4,128] one-bank PSUM tiles) on vector, then store all 4 windows
  in ONE [128,16,309] op (2.53MB contiguous HBM, same 4944B packets,
  quarter the dispatch/sem overhead), alternating the two HWDGE rings.
- Scalar ACTIVATE copies only in the ramp (the act-table load costs 1.3us
  on scalar but no static DMA); vector handles steady-state copies.
- insert_library_loads suppressed (no gpsimd-library instruction used),
  enable_partition_id=False.
- Known-unfixable variance: clean mode ~77.5-78.5us; single-slow-engine
  mode ~88-90us; global-HBM mode ~95-100us (environmental).

Measured same-day: baseline [80160, 102485, 88656, 79340]; F1-lineage
cleans 77.3-78.3; this kernel [87123, 78349, 89353, 88625] in degraded
weather with the best weather-normalized ramp (deficit 0.71MB vs 1.20
baseline-F1). Relative error 0.0 (exact).
"""

import numpy as np

import concourse.bacc as bacc
import concourse.bass as bass
import concourse.mybir as mybir
import concourse.tile as tile
from concourse import masks
from concourse.bass_utils import run_bass_kernel_spmd

B, C, S = 16, 4, 160000
FRAME, HOP = 2048, 512
NF = (S - FRAME) // HOP + 1          # 309 frames
NBLK = NF + FRAME // HOP - 1         # 312 blocks of 512 samples actually used
N_CORES = 8
B_PER = B // N_CORES                 # 2 batches per core
F32 = mybir.dt.float32


class _NoLibBacc(bacc.Bacc):
    """Bacc whose compile() skips gpsimd library loads (kernel uses none)."""

    def insert_library_loads(self):
        pass


def build_bass():
    nc = _NoLibBacc(None, target_bir_lowering=False, enable_partition_id=False)
    x = nc.dram_tensor("x", [B_PER, C, S], F32, kind="ExternalInput")
    out = nc.dram_tensor("out", [B_PER, C * FRAME, NF], F32, kind="ExternalOutput")

    with tile.TileContext(nc) as tc:
        with (
            tc.tile_pool(name="singles", bufs=1) as singles,
            tc.tile_pool(name="a", bufs=6) as a_pool,
            tc.tile_pool(name="a2", bufs=4) as a2_pool,
            tc.tile_pool(name="t2", bufs=6) as t2_pool,
            tc.tile_pool(name="oq", bufs=4) as oq_pool,
            tc.tile_pool(name="oqr", bufs=8) as oqr_pool,
            tc.tile_pool(name="ps", bufs=8, space=bass.MemorySpace.PSUM) as ps_pool,
        ):
            ident = singles.tile([128, 128], F32)
            nc.vector.memset(ident[:], 0.0)
            masks.make_identity(nc, ident[:], nomemset=True)

            n_main = 0
            n_store = 0
            for b in range(B_PER):
                for c in range(C):
                    slab_off = (b * C + c) * S

                    # jt0+jt1 in one op: am[p, k, r] = X2[128k + p, r]
                    am = a_pool.tile([128, 2, HOP], F32)
                    src = bass.AP(x, slab_off,
                                  [[HOP, 128], [128 * HOP, 2], [1, HOP]])
                    ld_eng = [nc.sync, nc.scalar][n_main % 2] if n_main < 4 \
                        else nc.gpsimd
                    ld_eng.dma_start(out=am[:], in_=src)
                    # jt2 tail: 56 blocks
                    pj2 = NBLK - 256  # 56
                    a3 = a2_pool.tile([128, HOP], F32)
                    src2 = bass.AP(x, slab_off + 256 * HOP, [[HOP, pj2], [1, HOP]])
                    n_main += 1
                    nc.gpsimd.dma_start(out=a3[:pj2], in_=src2)

                    if b == 0 and c == 0:
                        # Slab0: build the 4 store windows DIRECTLY from
                        # the PSUM transpose tiles (no t2 stage): each
                        # window is 3 scalar segment-copies (one per jt),
                        # each ready as soon as that jt's transposes land
                        # - two fewer semaphore hops before the first
                        # store hits the wire.
                        psts = []
                        for jt in range(3):
                            pj = 128 if jt < 2 else pj2
                            a_ap = a3[:pj2] if jt == 2 else am[:, jt]
                            pst = ps_pool.tile([128, 4, 128], F32)
                            for i in range(4):
                                nc.tensor.transpose(
                                    pst[:, i, :pj], a_ap[:pj, i::4],
                                    ident[:pj, :pj]
                                )
                            psts.append(pst)
                        for q in range(4):
                            oqr = oqr_pool.tile([128, 4, NF], F32)
                            # jt0 cols j in [q,128) -> window cols [0,128-q)
                            nc.scalar.copy(
                                out=oqr[:, :, : 128 - q],
                                in_=psts[0][:, :, q:128],
                            )
                            # jt1 cols j in [128,256) -> [128-q, 256-q)
                            nc.scalar.copy(
                                out=oqr[:, :, 128 - q : 256 - q],
                                in_=psts[1][:, :, :128],
                            )
                            # jt2 cols j in [256, q+309) -> [256-q, 309)
                            nc.scalar.copy(
                                out=oqr[:, :, 256 - q :],
                                in_=psts[2][:, :, : q + NF - 256],
                            )
                            dst = bass.AP(
                                out,
                                (b * C * FRAME + c * FRAME + q * HOP) * NF,
                                [[4 * NF, 128], [NF, 4], [1, NF]],
                            )
                            n_store += 1
                            nc.sync.dma_start(out=dst, in_=oqr[:])
                        continue

                    # T2[p, i, j] = X2[j, 4p + i]
                    t2 = t2_pool.tile([128, 4, NBLK], F32)
                    for jt in range(3):
                        pj = 128 if jt < 2 else pj2
                        a_ap = a3[:pj2] if jt == 2 else am[:, jt]
                        pst = ps_pool.tile([128, 4, 128], F32)
                        for i in range(4):
                            nc.tensor.transpose(
                                pst[:, i, :pj], a_ap[:pj, i::4], ident[:pj, :pj]
                            )
                        nc.vector.tensor_copy(
                            out=t2[:, :, jt * 128 : jt * 128 + pj],
                            in_=pst[:, :, :pj],
                        )

                    if b == 0 and c <= 1:
                        # Ramp slabs: store each q-window as soon as its
                        # copy lands, so stores overlap the read-only
                        # phase. Copies ride the otherwise-idle SCALAR
                        # engine (ACTIVATE): the vector engine then only
                        # does t2 copies during the ramp, and the tile
                        # scheduler cannot push these window copies behind
                        # other slabs' t2 work. Slab0 additionally splits
                        # each window copy at the jt2 boundary so the bulk
                        # [q,256) part starts before the jt2 tail lands.
                        for q in range(4):
                            oqr = oqr_pool.tile([128, 4, NF], F32)
                            if c == 0:
                                nA = 256 - q
                                nc.scalar.copy(
                                    out=oqr[:, :, :nA],
                                    in_=t2[:, :, q:256],
                                )
                                nc.scalar.copy(
                                    out=oqr[:, :, nA:],
                                    in_=t2[:, :, 256 : q + NF],
                                )
                            else:
                                nc.scalar.copy(
                                    out=oqr[:], in_=t2[:, :, q : q + NF]
                                )
                            dst = bass.AP(
                                out,
                                (b * C * FRAME + c * FRAME + q * HOP) * NF,
                                [[4 * NF, 128], [NF, 4], [1, NF]],
                            )
                            n_store += 1
                            nc.sync.dma_start(out=dst, in_=oqr[:])
                    else:
                        # One store op for all 4 q-windows: rows 512q+4p+i,
                        # covering the contiguous rows [c*2048, (c+1)*2048).
                        oq = oq_pool.tile([128, 4, 4, NF], F32)
                        for q in range(4):
                            nc.vector.tensor_copy(
                                out=oq[:, q], in_=t2[:, :, q : q + NF]
                            )
                        dst = bass.AP(
                            out,
                            (b * C * FRAME + c * FRAME) * NF,
                            [[4 * NF, 128], [HOP * NF, 4], [NF, 4], [1, NF]],
                        )
                        st_eng = [nc.scalar, nc.sync][n_store % 2]
                        n_store += 1
                        st_eng.dma_start(out=dst, in_=oq[:])
    nc.finalize()
    return nc


_NC_CACHE = None


def kernel(x: np.ndarray) -> np.ndarray:
    global _NC_CACHE
    if _NC_CACHE is None:
        _NC_CACHE = build_bass()
    nc = _NC_CACHE
    in_maps = [
        {"x": np.ascontiguousarray(x[i * B_PER : (i + 1) * B_PER])}
        for i in range(N_CORES)
    ]
    res = run_bass_kernel_spmd(nc, in_maps, list(range(N_CORES)))
    return np.concatenate([r["out"] for r in res.results], axis=0)
